# revision 1
# baseline (speedup 1.0000x reference)
"""ImageStreamBlock kernel for 8 Trainium2 NeuronCores.

Distribution strategy (per spec sharding_hint): tensor-parallel across
attention heads for the qkv/attention stage (2 heads per core), with the
modulation / LayerNorm / MLP stages computed on-device as well. Inputs
are accepted FULL-shape; sharding happens inside kernel(); the full
output is reassembled before returning.

Self-contained: hardcodes all shapes from the problem spec.
"""

import numpy as np
import jax
import jax.numpy as jnp
from functools import partial

# Problem shapes (hardcoded per spec nn_ImageStreamBlock_30623116821345)
B, LI, LT = 1, 2048, 256
H, NH, HD, MLP = 1024, 16, 64, 4096
L = LI + LT
NCORES = 8
HPC = NH // NCORES  # heads per core


def _ln(x):
    m = jnp.mean(x, -1, keepdims=True)
    v = jnp.mean((x - m) ** 2, -1, keepdims=True)
    return (x - m) * jax.lax.rsqrt(v + 1e-6)


def _rms(x, scale):
    return x * jax.lax.rsqrt(jnp.mean(x * x, -1, keepdims=True) + 1e-6) * scale


def _rope(x, pe):
    # x: [B, h, L, HD]; pe: [1, 1, L, HD//2, 2, 2]
    xr = x.reshape(x.shape[:-1] + (HD // 2, 1, 2))
    out = pe[..., 0] * xr[..., 0] + pe[..., 1] * xr[..., 1]
    return out.reshape(x.shape)


@partial(jax.jit, static_argnums=())
def _stage1(img, txt, vec, img_mod_w, img_mod_b, txt_mod_w, txt_mod_b,
            img_qkv_w, img_qkv_b, txt_qkv_w, txt_qkv_b,
            img_q_scale, img_k_scale, txt_q_scale, txt_k_scale, pe):
    """Everything up to per-head q,k,v (replicated part is cheap)."""
    sv = jax.nn.silu(vec)
    imod = (jnp.einsum('bd,de->be', sv, img_mod_w) + img_mod_b)[:, None, :]
    i_sh1, i_sc1, i_g1, i_sh2, i_sc2, i_g2 = jnp.split(imod, 6, -1)
    tmod = (jnp.einsum('bd,de->be', sv, txt_mod_w) + txt_mod_b)[:, None, :]
    t_sh1, t_sc1 = jnp.split(tmod, 6, -1)[0], jnp.split(tmod, 6, -1)[1]

    img_m = (1 + i_sc1) * _ln(img) + i_sh1
    txt_m = (1 + t_sc1) * _ln(txt) + t_sh1

    def qkv_heads(x, w, b):
        b_, l_, _ = x.shape
        qkv = (jnp.einsum('bld,de->ble', x, w) + b).reshape(b_, l_, 3, NH, HD)
        qkv = jnp.transpose(qkv, (2, 0, 3, 1, 4))
        return qkv[0], qkv[1], qkv[2]

    iq, ik, iv = qkv_heads(img_m, img_qkv_w, img_qkv_b)
    iq, ik = _rms(iq, img_q_scale), _rms(ik, img_k_scale)
    tq, tk, tv = qkv_heads(txt_m, txt_qkv_w, txt_qkv_b)
    tq, tk = _rms(tq, txt_q_scale), _rms(tk, txt_k_scale)

    q = _rope(jnp.concatenate([tq, iq], 2), pe)
    k = _rope(jnp.concatenate([tk, ik], 2), pe)
    v = jnp.concatenate([tv, iv], 2)
    return q, k, v, i_sh2, i_sc2, i_g1, i_g2


@jax.jit
def _attn_heads(q, k, v):
    """Attention for a slice of heads: q,k,v [B, h, L, HD] -> [B, h, L, HD]."""
    scores = jnp.einsum('bhqd,bhkd->bhqk', q, k) / np.float32(np.sqrt(HD))
    return jnp.einsum('bhqk,bhkd->bhqd', jax.nn.softmax(scores, -1), v)


@jax.jit
def _proj_part(attn_h, proj_w_rows):
    """Partial projection: attn slice [B, LI, HPC*HD] @ rows of proj_w."""
    return jnp.einsum('bld,de->ble', attn_h, proj_w_rows)


@jax.jit
def _stage3(img, proj_sum, img_proj_b, i_g1, i_sh2, i_sc2, i_g2,
            mlp_w1, mlp_b1, mlp_w2, mlp_b2):
    img2 = img + i_g1 * (proj_sum + img_proj_b)
    h = (1 + i_sc2) * _ln(img2) + i_sh2
    h = jax.nn.gelu(jnp.einsum('bld,de->ble', h, mlp_w1) + mlp_b1,
                    approximate=True)
    h = jnp.einsum('bld,de->ble', h, mlp_w2) + mlp_b2
    return img2 + i_g2 * h


@jax.jit
def _mlp_part(img2_rows, i_sh2_, i_sc2_, mlp_w1, mlp_b1, mlp_w2, mlp_b2):
    h = (1 + i_sc2_) * _ln(img2_rows) + i_sh2_
    h = jax.nn.gelu(jnp.einsum('bld,de->ble', h, mlp_w1) + mlp_b1,
                    approximate=True)
    return jnp.einsum('bld,de->ble', h, mlp_w2) + mlp_b2


def kernel(**inputs: np.ndarray) -> np.ndarray:
    devs = jax.devices()[:NCORES]

    img = inputs['img']
    txt = inputs['txt']
    pe = inputs['pe']

    # ---- stage 1 (replicated trunk on core 0: mod + LN + qkv + rope) ----
    d0 = devs[0]
    put0 = lambda x: jax.device_put(np.asarray(x), d0)
    q, k, v, i_sh2, i_sc2, i_g1, i_g2 = _stage1(
        put0(img), put0(txt), put0(inputs['vec']),
        put0(inputs['img_mod_w']), put0(inputs['img_mod_b']),
        put0(inputs['txt_mod_w']), put0(inputs['txt_mod_b']),
        put0(inputs['img_qkv_w']), put0(inputs['img_qkv_b']),
        put0(inputs['txt_qkv_w']), put0(inputs['txt_qkv_b']),
        put0(inputs['img_q_scale']), put0(inputs['img_k_scale']),
        put0(inputs['txt_q_scale']), put0(inputs['txt_k_scale']),
        put0(pe),
    )
    q = np.asarray(q); k = np.asarray(k); v = np.asarray(v)

    # ---- stage 2: attention, tensor-parallel across heads (2 per core) ----
    proj_w = inputs['img_proj_w']
    attn_parts = []
    for c in range(NCORES):
        hs = slice(c * HPC, (c + 1) * HPC)
        qc = jax.device_put(q[:, hs], devs[c])
        kc = jax.device_put(k[:, hs], devs[c])
        vc = jax.device_put(v[:, hs], devs[c])
        attn_parts.append(_attn_heads(qc, kc, vc))

    # partial proj on each core over its head-feature rows
    proj_parts = []
    for c in range(NCORES):
        a = attn_parts[c]                      # [B, HPC, L, HD] on dev c
        a_img = jnp.transpose(a, (0, 2, 1, 3)).reshape(1, L, HPC * HD)[:, LT:]
        w_rows = jax.device_put(
            proj_w[c * HPC * HD:(c + 1) * HPC * HD], devs[c])
        proj_parts.append(_proj_part(a_img, w_rows))

    proj_sum = np.zeros((1, LI, H), np.float32)
    for c in range(NCORES):
        proj_sum += np.asarray(proj_parts[c])

    # ---- stage 3: residual + LN2 + MLP, data-parallel over tokens ----
    rows = LI // NCORES
    i_sh2_n = np.asarray(i_sh2); i_sc2_n = np.asarray(i_sc2)
    i_g1_n = np.asarray(i_g1); i_g2_n = np.asarray(i_g2)
    img2 = img + i_g1_n * (proj_sum + inputs['img_proj_b'])
    out_parts = []
    for c in range(NCORES):
        sl = slice(c * rows, (c + 1) * rows)
        out_parts.append(_mlp_part(
            jax.device_put(img2[:, sl], devs[c]),
            jax.device_put(i_sh2_n, devs[c]),
            jax.device_put(i_sc2_n, devs[c]),
            jax.device_put(inputs['mlp_w1'], devs[c]),
            jax.device_put(inputs['mlp_b1'], devs[c]),
            jax.device_put(inputs['mlp_w2'], devs[c]),
            jax.device_put(inputs['mlp_b2'], devs[c]),
        ))
    h_full = np.concatenate([np.asarray(p) for p in out_parts], axis=1)
    out = img2 + i_g2_n * h_full
    return out.astype(np.float32)
